# revision 10
# baseline (speedup 1.0000x reference)
"""Trainium2 Bass kernel for nn_BoundarySeg (segment_reduce).

out[b, j, 0:H]   = sum_{i>=j} A[b, j, i] * h[b, i, :]
out[b, j, H:2H]  = h[b, j, :] * sum_{i>=j} A[b, j, i]

Shapes: A [8, 2048, 2048] f32, h [8, 2048, 256] f32 -> out [8, 2048, 512] f32.
Sharding: data-parallel over batch; core c computes batch c.

Per-core algorithm (L=2048 in 16 tiles of 128, H=256), bf16 pipeline:
  - Panels process in DESCENDING order (15 -> 0): the small tail panels only
    need the high h tiles, so the pipeline fills while most of h and the big
    panels are still in flight; the biggest panel finishes last, right after
    its (late) load.
  - A loads as 9 f32 rectangles (rows [jc, jc+nr) x cols [jc, NT); the
    second row over-fetches one block below the diagonal) split across the
    two HWDGE rings in consumption order; h loads as four 0.5MB quarters
    woven between the A units on whichever ring is lighter.
  - Per panel: one whole-panel f32->bf16 cast (ACT), per-block TensorE
    transposes in bf16 (1 cyc/row), then PSUM->SBUF: a masked tensor_tensor
    on the diagonal block (keep i >= j) plus one plain 2x copy (DVE).
  - acc[j, n] += atT_block^T @ h_ext over i-tiles >= jc (bf16 x bf16 into
    f32 PSUM, N=258; col 256 of h_ext is ones so the row-sum falls out as an
    extra column); panel pairs share one 2-bank PSUM acc tile.
  - Outputs are bf16 (tolerance 2e-2; host casts back): first half is one
    fused PSUM->SBUF cast per pair (ACT/DVE alternating), second half is a
    DVE tensor_scalar of h_ext by the rowsum column read straight from PSUM;
    stores stream on the SWDGE (gpsimd) queue as two DMAs per pair.
"""

import os
import sys

import numpy as np

sys.path.insert(0, "/opt/trn_rl_repo")

import concourse.bass as bass  # noqa: E402
import concourse.bacc as bacc  # noqa: E402
import concourse.tile as tile  # noqa: E402
from concourse import mybir  # noqa: E402
from concourse.bass_utils import run_bass_kernel_spmd  # noqa: E402
from concourse.masks import make_identity, make_lower_triangular  # noqa: E402

B, L, H = 8, 2048, 256
P = 128
NT = 16
HE = H + 2  # col H = ones (rowsum), col H+1 unused

F32 = mybir.dt.float32
BF16 = mybir.dt.bfloat16

# (first row-tile, n row-tiles, ring) in processing order (descending).
# ring 0 = sync HWDGE, ring 1 = scalar HWDGE.
UNITS = [
    (14, 2, 0),
    (12, 2, 0),
    (10, 2, 1),
    (8, 2, 0),
    (6, 2, 1),
    (4, 2, 0),
    (2, 2, 1),
    (0, 1, 0),
    (1, 1, 1),
]
# h quarters: (tile range, ring, emit before unit index)
H_LOADS = [
    ((12, 16), 1, 0),
    ((8, 12), 1, 0),
    ((4, 8), 0, 2),
    ((0, 4), 0, 4),
]

LAST_RESULTS = None
_NC_CACHE = {}


def _build_nc():
    nc = bacc.Bacc(None, target_bir_lowering=False)
    a_dram = nc.dram_tensor("a", [L, L], F32, kind="ExternalInput")
    h_dram = nc.dram_tensor("h", [L, H], F32, kind="ExternalInput")
    out_dram = nc.dram_tensor("out", [L, 2 * H], BF16, kind="ExternalOutput")

    with tile.TileContext(nc) as tc:
        with (
            tc.tile_pool(name="const", bufs=1) as const_pool,
            tc.tile_pool(name="hpool", bufs=1) as h_pool,
            tc.tile_pool(name="astage", bufs=4) as a_pool,
            tc.tile_pool(name="abf", bufs=4) as ab_pool,
            tc.tile_pool(name="atT", bufs=4) as at_pool,
            tc.tile_pool(name="tp", bufs=2, space=bass.MemorySpace.PSUM) as tp_pool,
            tc.tile_pool(name="acc", bufs=2, space=bass.MemorySpace.PSUM) as acc_pool,
            tc.tile_pool(name="out1", bufs=3) as o1_pool,
            tc.tile_pool(name="out2", bufs=3) as o2_pool,
        ):
            identity = const_pool.tile([P, P], BF16)
            make_identity(nc, identity[:])
            # Mask for the transposed diagonal block ([i(part), j(free)]):
            # keep i >= j -> lower triangular incl diag.
            cmask = const_pool.tile([P, P], BF16)
            make_lower_triangular(nc, cmask[:], val=1.0, diag=True)

            h_stage = h_pool.tile([P, NT, H], F32)
            h_ext = h_pool.tile([P, NT, HE], BF16)
            h_re = h_dram[:].rearrange("(t p) n -> p t n", p=P)
            nc.vector.memset(h_ext[:, :, H:HE], 1.0)
            ring = [nc.sync, nc.scalar]

            def load_h_quarter(qi):
                (ta, tb), rg, _ = H_LOADS[qi]
                ring[rg].dma_start(
                    out=h_stage[:, ta:tb, :], in_=h_re[:, ta:tb, :]
                )
                if qi % 2 == 0:
                    nc.scalar.copy(h_ext[:, ta:tb, 0:H], h_stage[:, ta:tb, :])
                else:
                    nc.vector.tensor_copy(h_ext[:, ta:tb, 0:H], h_stage[:, ta:tb, :])

            # Warmup transpose: absorbs the Pool->PE wait for `identity`.
            wtp = tp_pool.tile([P, NT * P], BF16, tag="tp")
            nc.tensor.transpose(wtp[:, 0:P], identity[:], identity[:])

            state = {}

            def matmuls_and_store(jc, atT):
                ntiles = NT - jc
                t = jc % 2
                if t == 0:
                    acc_t = acc_pool.tile([P, 2, 512], F32, tag="acc")
                    o1_t = o1_pool.tile([P, 2, H], BF16, tag="out1")
                    o2_t = o2_pool.tile([P, 2, H], BF16, tag="out2")
                    state["acc"], state["o1"], state["o2"] = acc_t, o1_t, o2_t
                acc, o1, o2 = state["acc"], state["o1"], state["o2"]
                for k in range(ntiles):
                    nc.tensor.matmul(
                        acc[:, t, 0:HE],
                        atT[:, k * P : (k + 1) * P],
                        h_ext[:, jc + k, :],
                        start=(k == 0),
                        stop=(k == ntiles - 1),
                    )
                # second half: h[j,:] * rowsum (rowsum read straight from
                # PSUM as the tensor_scalar per-partition scalar).
                nc.vector.tensor_scalar(
                    o2[:, t, :],
                    h_ext[:, jc, 0:H],
                    acc[:, t, H : H + 1],
                    None,
                    mybir.AluOpType.mult,
                )
                if t == 1:
                    # first half for both panels in one fused PSUM read.
                    if (jc // 2) % 2 == 0:
                        nc.scalar.copy(o1[:], acc[:, :, 0:H])
                    else:
                        nc.vector.tensor_copy(o1[:], acc[:, :, 0:H])
                    ra, rb = (jc - 1) * P, (jc + 1) * P
                    nc.gpsimd.dma_start(
                        out_dram[ra:rb, 0:H].rearrange("(t p) n -> p t n", p=P),
                        o1[:],
                    )
                    nc.gpsimd.dma_start(
                        out_dram[ra:rb, H : 2 * H].rearrange("(t p) n -> p t n", p=P),
                        o2[:],
                    )

            hq_emitted = 0
            pending = []
            for ui, (r0, nr, rg) in enumerate(UNITS):
                while hq_emitted < 4 and H_LOADS[hq_emitted][2] <= ui:
                    load_h_quarter(hq_emitted)
                    hq_emitted += 1
                w_u = NT - r0
                src = a_dram[r0 * P : (r0 + nr) * P, r0 * P :].rearrange(
                    "(t p) w -> p t w", p=P
                )
                a_stage = a_pool.tile([P, nr, w_u * P], F32, tag="astage")
                ring[rg].dma_start(a_stage[:], src)

                # within a unit, panels still go even-first so acc pairs align
                for t in range(nr):
                    jc = r0 + t
                    w_jc = NT - jc
                    skip = jc - r0
                    panel_src = a_stage[:, t, skip * P : (skip + w_jc) * P]
                    ab = ab_pool.tile([P, NT * P], BF16, tag="abf")
                    nc.scalar.copy(ab[:, 0 : w_jc * P], panel_src)
                    tp = tp_pool.tile([P, NT * P], BF16, tag="tp")
                    for k in range(w_jc):
                        nc.tensor.transpose(
                            tp[:, k * P : (k + 1) * P],
                            ab[:, k * P : (k + 1) * P],
                            identity[:],
                        )
                    atT = at_pool.tile([P, w_jc * P], BF16, tag="atT")
                    nc.vector.tensor_tensor(
                        atT[:, 0:P], tp[:, 0:P], cmask[:], mybir.AluOpType.mult
                    )
                    if w_jc > 1:
                        nc.vector.tensor_copy(
                            atT[:, P : w_jc * P], tp[:, P : w_jc * P]
                        )
                    pending.append((jc, atT))
                    if len(pending) > 2:
                        matmuls_and_store(*pending.pop(0))

            for item in pending:
                matmuls_and_store(*item)

    nc.finalize()
    return nc


def kernel(span_adjacency, bound_hidden):
    global LAST_RESULTS
    a = np.ascontiguousarray(np.asarray(span_adjacency, dtype=np.float32))
    h = np.ascontiguousarray(np.asarray(bound_hidden, dtype=np.float32))
    assert a.shape == (B, L, L) and h.shape == (B, L, H), (a.shape, h.shape)

    key = "full"
    if key not in _NC_CACHE:
        _NC_CACHE[key] = _build_nc()
    nc = _NC_CACHE[key]

    in_maps = [{"a": a[b], "h": h[b]} for b in range(B)]
    res = run_bass_kernel_spmd(
        nc,
        in_maps,
        core_ids=list(range(B)),
        trace=bool(os.environ.get("KERNEL_TRACE")),
    )
    LAST_RESULTS = res
    out = np.stack(
        [np.asarray(res.results[b]["out"]).astype(np.float32) for b in range(B)],
        axis=0,
    )
    return out


# revision 12
# speedup vs baseline: 1.0650x; 1.0650x over previous
"""Trainium2 Bass kernel for nn_BoundarySeg (segment_reduce).

out[b, j, 0:H]   = sum_{i>=j} A[b, j, i] * h[b, i, :]
out[b, j, H:2H]  = h[b, j, :] * sum_{i>=j} A[b, j, i]

Shapes: A [8, 2048, 2048] f32, h [8, 2048, 256] f32 -> out [8, 2048, 512] f32.
Sharding: data-parallel over batch; core c computes batch c.

Per-core algorithm (L=2048 in 16 tiles of 128, H=256), bf16 pipeline:
  - Panels process in DESCENDING order (15 -> 0): the small tail panels only
    need the high h tiles, so the pipeline fills while most of h and the big
    panels are still in flight; the biggest panel finishes last, right after
    its (late) load.
  - A loads as 9 f32 rectangles (rows [jc, jc+nr) x cols [jc, NT); the
    second row over-fetches one block below the diagonal) split across the
    two HWDGE rings in consumption order; h loads as four 0.5MB quarters
    woven between the A units on whichever ring is lighter.
  - Per panel: one whole-panel f32->bf16 cast (ACT), per-block TensorE
    transposes in bf16 (1 cyc/row), then PSUM->SBUF: a masked tensor_tensor
    on the diagonal block (keep i >= j) plus one plain 2x copy (DVE).
  - acc[j, n] += atT_block^T @ h_ext over i-tiles >= jc (bf16 x bf16 into
    f32 PSUM, N=258; col 256 of h_ext is ones so the row-sum falls out as an
    extra column); panel pairs share one 2-bank PSUM acc tile.
  - Outputs are bf16 (tolerance 2e-2; host casts back): first half is one
    fused PSUM->SBUF cast per pair (ACT/DVE alternating), second half is a
    DVE tensor_scalar of h_ext by the rowsum column read straight from PSUM;
    stores stream on the SWDGE (gpsimd) queue as two DMAs per pair.
"""

import os
import sys

import numpy as np

sys.path.insert(0, "/opt/trn_rl_repo")

import concourse.bass as bass  # noqa: E402
import concourse.bacc as bacc  # noqa: E402
import concourse.tile as tile  # noqa: E402
from concourse import mybir  # noqa: E402
from concourse.bass_utils import run_bass_kernel_spmd  # noqa: E402
from concourse.masks import make_identity, make_lower_triangular  # noqa: E402

B, L, H = 8, 2048, 256
P = 128
NT = 16
HE = H + 2  # col H = ones (rowsum), col H+1 unused

F32 = mybir.dt.float32
BF16 = mybir.dt.bfloat16

# (first row-tile, n row-tiles, ring) in processing order: small panels
# first (they only need the high h tiles and fill the pipe fast), then the
# big panels ascending so the tail chain is short.
# ring 0 = sync HWDGE, ring 1 = scalar HWDGE.
UNITS = [
    (14, 2, 0),
    (12, 2, 0),
    (10, 2, 1),
    (8, 2, 0),
    (0, 1, 0),
    (1, 1, 1),
    (2, 2, 0),
    (4, 2, 1),
    (6, 2, 0),
]
# h quarters: (tile range, ring, emit before unit index) — high half first.
H_LOADS = [
    ((12, 16), 1, 0),
    ((8, 12), 1, 0),
    ((0, 4), 1, 3),
    ((4, 8), 1, 4),
]

LAST_RESULTS = None
_NC_CACHE = {}


def _build_nc():
    nc = bacc.Bacc(None, target_bir_lowering=False)
    a_dram = nc.dram_tensor("a", [L, L], F32, kind="ExternalInput")
    h_dram = nc.dram_tensor("h", [L, H], F32, kind="ExternalInput")
    out_dram = nc.dram_tensor("out", [L, 2 * H], BF16, kind="ExternalOutput")

    with tile.TileContext(nc) as tc:
        with (
            tc.tile_pool(name="const", bufs=1) as const_pool,
            tc.tile_pool(name="hpool", bufs=1) as h_pool,
            tc.tile_pool(name="astage", bufs=9) as a_pool,
            tc.tile_pool(name="abf", bufs=4) as ab_pool,
            tc.tile_pool(name="atT", bufs=4) as at_pool,
            tc.tile_pool(name="tp", bufs=2, space=bass.MemorySpace.PSUM) as tp_pool,
            tc.tile_pool(name="acc", bufs=2, space=bass.MemorySpace.PSUM) as acc_pool,
            tc.tile_pool(name="out1", bufs=3) as o1_pool,
            tc.tile_pool(name="out2", bufs=3) as o2_pool,
        ):
            identity = const_pool.tile([P, P], BF16)
            make_identity(nc, identity[:])
            # Mask for the transposed diagonal block ([i(part), j(free)]):
            # keep i >= j -> lower triangular incl diag.
            cmask = const_pool.tile([P, P], BF16)
            make_lower_triangular(nc, cmask[:], val=1.0, diag=True)

            h_stage = h_pool.tile([P, NT, H], F32)
            h_ext = h_pool.tile([P, NT, HE], BF16)
            h_re = h_dram[:].rearrange("(t p) n -> p t n", p=P)
            nc.vector.memset(h_ext[:, :, H:HE], 1.0)
            ring = [nc.sync, nc.scalar]

            def load_h_quarter(qi):
                (ta, tb), rg, _ = H_LOADS[qi]
                ring[rg].dma_start(
                    out=h_stage[:, ta:tb, :], in_=h_re[:, ta:tb, :]
                )
                if qi % 2 == 0:
                    nc.scalar.copy(h_ext[:, ta:tb, 0:H], h_stage[:, ta:tb, :])
                else:
                    nc.vector.tensor_copy(h_ext[:, ta:tb, 0:H], h_stage[:, ta:tb, :])

            # Warmup transpose: absorbs the Pool->PE wait for `identity`.
            wtp = tp_pool.tile([P, NT * P], BF16, tag="tp")
            nc.tensor.transpose(wtp[:, 0:P], identity[:], identity[:])

            state = {}

            def matmuls_and_store(jc, atT):
                ntiles = NT - jc
                t = jc % 2
                if t == 0:
                    acc_t = acc_pool.tile([P, 2, 512], F32, tag="acc")
                    o1_t = o1_pool.tile([P, 2, H], BF16, tag="out1")
                    o2_t = o2_pool.tile([P, 2, H], BF16, tag="out2")
                    state["acc"], state["o1"], state["o2"] = acc_t, o1_t, o2_t
                acc, o1, o2 = state["acc"], state["o1"], state["o2"]
                for k in range(ntiles):
                    nc.tensor.matmul(
                        acc[:, t, 0:HE],
                        atT[:, k * P : (k + 1) * P],
                        h_ext[:, jc + k, :],
                        start=(k == 0),
                        stop=(k == ntiles - 1),
                    )
                # second half: h[j,:] * rowsum (rowsum read straight from
                # PSUM as the tensor_scalar per-partition scalar).
                nc.vector.tensor_scalar(
                    o2[:, t, :],
                    h_ext[:, jc, 0:H],
                    acc[:, t, H : H + 1],
                    None,
                    mybir.AluOpType.mult,
                )
                if t == 1:
                    # first half for both panels in one fused PSUM read.
                    if (jc // 2) % 2 == 0:
                        nc.scalar.copy(o1[:], acc[:, :, 0:H])
                    else:
                        nc.vector.tensor_copy(o1[:], acc[:, :, 0:H])
                    ra, rb = (jc - 1) * P, (jc + 1) * P
                    nc.gpsimd.dma_start(
                        out_dram[ra:rb, 0:H].rearrange("(t p) n -> p t n", p=P),
                        o1[:],
                    )
                    nc.gpsimd.dma_start(
                        out_dram[ra:rb, H : 2 * H].rearrange("(t p) n -> p t n", p=P),
                        o2[:],
                    )

            hq_emitted = 0
            pending = []
            for ui, (r0, nr, rg) in enumerate(UNITS):
                while hq_emitted < 4 and H_LOADS[hq_emitted][2] <= ui:
                    load_h_quarter(hq_emitted)
                    hq_emitted += 1
                w_u = NT - r0
                src = a_dram[r0 * P : (r0 + nr) * P, r0 * P :].rearrange(
                    "(t p) w -> p t w", p=P
                )
                a_stage = a_pool.tile([P, nr, w_u * P], F32, tag="astage")
                ring[rg].dma_start(a_stage[:], src)

                # within a unit, panels still go even-first so acc pairs align
                for t in range(nr):
                    jc = r0 + t
                    w_jc = NT - jc
                    skip = jc - r0
                    panel_src = a_stage[:, t, skip * P : (skip + w_jc) * P]
                    ab = ab_pool.tile([P, NT * P], BF16, tag="abf")
                    nc.scalar.copy(ab[:, 0 : w_jc * P], panel_src)
                    tp = tp_pool.tile([P, NT * P], BF16, tag="tp")
                    for k in range(w_jc):
                        nc.tensor.transpose(
                            tp[:, k * P : (k + 1) * P],
                            ab[:, k * P : (k + 1) * P],
                            identity[:],
                        )
                    atT = at_pool.tile([P, w_jc * P], BF16, tag="atT")
                    nc.vector.tensor_tensor(
                        atT[:, 0:P], tp[:, 0:P], cmask[:], mybir.AluOpType.mult
                    )
                    if w_jc > 1:
                        nc.vector.tensor_copy(
                            atT[:, P : w_jc * P], tp[:, P : w_jc * P]
                        )
                    pending.append((jc, atT))
                    if len(pending) > 2:
                        matmuls_and_store(*pending.pop(0))

            for item in pending:
                matmuls_and_store(*item)

    nc.finalize()
    return nc


def kernel(span_adjacency, bound_hidden):
    global LAST_RESULTS
    a = np.ascontiguousarray(np.asarray(span_adjacency, dtype=np.float32))
    h = np.ascontiguousarray(np.asarray(bound_hidden, dtype=np.float32))
    assert a.shape == (B, L, L) and h.shape == (B, L, H), (a.shape, h.shape)

    key = "full"
    if key not in _NC_CACHE:
        _NC_CACHE[key] = _build_nc()
    nc = _NC_CACHE[key]

    in_maps = [{"a": a[b], "h": h[b]} for b in range(B)]
    res = run_bass_kernel_spmd(
        nc,
        in_maps,
        core_ids=list(range(B)),
        trace=bool(os.environ.get("KERNEL_TRACE")),
    )
    LAST_RESULTS = res
    out = np.stack(
        [np.asarray(res.results[b]["out"]).astype(np.float32) for b in range(B)],
        axis=0,
    )
    return out
